# revision 6
# baseline (speedup 1.0000x reference)
"""BallQueryAttention TRN2 kernel, v14 = v13 + uniform 2b-s convention + 3 G buffers.

G[j,i] = hi_j . hi_i + v(i) with K=66 (64 hi dims + two ones-rows carrying
v1+v2 = fp16 split of -0.5*sq_i).  Drops all hi/lo cross terms: ~4e-3 rms on
d2 -> ~2e-3 L2 output error (gate 2e-2).  2 standalone MMs per G tile.

Same math as v2 (see kernel_v2.py docstring) with a restructured preamble:

  - 4 j-side column groups (8/18/19/19 tiles) instead of 16: far fewer
    DMA-transpose serialization points; group 0 is small so the main loop
    starts early.
  - persistent HILO [128, JT*128] staging is the pass-C stationary directly
    (layout [hi(64) | 1 | lo(0:62) | 1] per tile), eliminating the separate
    XW copy.
  - the two '1' stationary rows sit at partitions 64 and 127; pass A rides
    v1/v2 there, pass B rides hi62/hi63 (cancelled by v).
  - i-side staging chain runs at high scheduler priority; its DRAM writes
    and MA/MB transposes are queued before all j-side DMA traffic.
"""

import sys

sys.path.insert(0, "/opt/trn_rl_repo")

import numpy as np

import concourse.bass as bass
import concourse.tile as tile
from concourse import bacc, masks, mybir
from concourse.bass_utils import run_bass_kernel_spmd

F32 = mybir.dt.float32
F16 = mybir.dt.float16
AF = mybir.ActivationFunctionType
OP = mybir.AluOpType

N = 8192
D = 64
NCORES = 8
ROWS = N // NCORES          # 1024 rows per core
JT = N // 128               # 64 j-tiles
IT = ROWS // 128            # 8 i-tiles
GROUPS = [8, 18, 19, 19]    # j-tiles per preamble group
R2 = 11.0 * 11.0
K1 = (np.e - 1.0) / 2.0

FLAGS = {
    "compare": True,
    "gmm": True,
    "passc": True,
    "lag": 3,
}


def _pre(nc, tc, pools, xf, xi, outd, dram):
    const, scratch, gpool, mpool, apool, spool = pools
    ts = bass.ts

    # ---------------- persistent tiles ----------------
    W1g = [const.tile([128, nt * 128], F16, name=f"W1_{g}", tag=f"W1_{g}")
           for g, nt in enumerate((8, 18, 38))]
    XW = const.tile([128, 65 * JT], F16, tag="XW")
    XW2 = const.tile([128, 65 * JT], F16, tag="XW2")
    SPART = const.tile([128, D], F32, tag="SPART")
    MAB = const.tile([128, ROWS], F16, tag="MAB")   # MA only (single pass)
    biasA = const.tile([128, JT], F32, tag="biasA")
    thrD = const.tile([128, JT], F32, tag="thrD")
    ONEC = const.tile([128, 1], F16, tag="ONEC")
    IDN = const.tile([65, 65], F32, tag="IDN")

    nc.vector.memset(ONEC[:], 1.0)
    masks.make_identity(nc, IDN[:])

    # ---------------- preamble: i side (high priority) ----------------
    rABd = dram.tile([ROWS, 128], F16, tag="rABd")

    with tc.high_priority():
        xitp = scratch.tile([128, IT * D], F32, tag="xitp")  # row p*IT + t
        nc.gpsimd.dma_start(xitp[:], xi.rearrange("(p t) d -> p (t d)", p=128))
        xitp3 = xitp[:].rearrange("p (t d) -> p t d", d=D)

        rA = scratch.tile([128, IT * 128], F16, tag="rA")
        rA3 = rA[:].rearrange("p (t e) -> p t e", e=128)
        nc.vector.tensor_copy(rA3[:, :, 0:D], xitp3)                    # hi
        nc.gpsimd.memset(rA3[:, :, D + 2 : 128], 0.0)                   # unused

        # v = -0.5*sq_i, split v -> v1 + v2 (fp16) at cols 64, 65
        s2i = scratch.tile([128, IT * D], F32, tag="s2i")
        nc.scalar.activation(s2i[:], xitp[:], AF.Square)
        sqit = scratch.tile([128, IT], F32, tag="sqit")
        nc.vector.tensor_reduce(sqit[:],
                                s2i[:].rearrange("p (t d) -> p t d", d=D),
                                axis=mybir.AxisListType.X, op=OP.add)
        vfull = scratch.tile([128, IT], F32, tag="vfull")
        nc.vector.tensor_scalar(vfull[:], sqit[:], -0.5, None, OP.mult)
        v1 = scratch.tile([128, IT], F16, tag="v1")
        nc.vector.tensor_copy(v1[:], vfull[:])
        rv1 = scratch.tile([128, IT], F32, tag="rv1")
        nc.vector.tensor_tensor(rv1[:], vfull[:], v1[:], OP.subtract)
        v2 = scratch.tile([128, IT], F16, tag="v2")
        nc.vector.tensor_copy(v2[:], rv1[:])
        nc.vector.tensor_copy(rA3[:, :, D : D + 1],
                              v1[:].rearrange("p (t u) -> p t u", u=1))
        nc.vector.tensor_copy(rA3[:, :, D + 1 : D + 2],
                              v2[:].rearrange("p (t u) -> p t u", u=1))

        nc.scalar.dma_start(
            rABd[0:ROWS, :].rearrange("(p t) e -> p (t e)", p=128), rA[:])
        nc.sync.dma_start(MAB[:, 0:ROWS], rABd[0:ROWS, :], transpose=True)

        # trigger the Sign act-table load early so it overlaps the preamble
        dumm = spool.tile([128, 1], F32, tag="dumm")
        nc.scalar.activation(dumm[:], xitp[:, 0:1], AF.Sign)

    # ---------------- preamble: j side ----------------
    hilod = dram.tile([N, 128], F16, tag="hilod")
    t0 = 0
    for g, nt in enumerate(GROUPS):
        gsl = slice(t0, t0 + nt)
        xtp = scratch.tile([128, nt * D], F32, name=f"xtp{g}", tag=f"xtp{g}")
        # j-tile t holds rows {p*64 + t}: contiguous per-partition load; the
        # relabeling is invisible (j only ever summed over)
        nc.gpsimd.dma_start(
            xtp[:].rearrange("p (t d) -> p t d", d=D),
            xf.rearrange("(p t) d -> p t d", p=128)[:, gsl, :],
        )
        xtp3 = xtp[:].rearrange("p (t d) -> p t d", d=D)

        xw3 = XW[:].rearrange("p (t e) -> p t e", e=65)[:, gsl, :]
        nc.vector.tensor_copy(xw3[:, :, 0:D], xtp3)
        nc.gpsimd.memset(xw3[:, :, D : D + 1], 1.0)
        xw23 = XW2[:].rearrange("p (t e) -> p t e", e=65)[:, gsl, :]
        nc.vector.tensor_scalar(xw23[:, :, 0:D], xtp3, 2.0, None, OP.mult)
        nc.gpsimd.memset(xw23[:, :, D : D + 1], 2.0)
        # f32 colsum partial over tiles (contiguous [128, 64] adds)
        for tt_ in range(nt):
            if t0 + tt_ == 0:
                nc.vector.tensor_copy(SPART[:], xtp3[:, 0, :])
            else:
                nc.vector.tensor_tensor(SPART[:], SPART[:], xtp3[:, tt_, :],
                                        OP.add)

        hilo = scratch.tile([128, nt * 128], F16, name=f"hilo{g}", tag=f"hilo{g}")
        hg = hilo[:].rearrange("p (t e) -> p t e", e=128)
        nc.vector.tensor_copy(hg[:, :, 0:D], xtp3)                      # hi_j
        nc.gpsimd.memset(hg[:, :, D : D + 2], 1.0)                      # ones rows
        nc.gpsimd.memset(hg[:, :, D + 2 : 128], 0.0)                    # unused
        hseg = hilod[t0 * 128 : (t0 + nt) * 128, :]
        nc.gpsimd.dma_start(hseg.rearrange("(t p) e -> p t e", p=128), hg)
        if g <= 1:
            nc.sync.dma_start(W1g[g][:], hseg, transpose=True)
        elif g == 3:
            nc.sync.dma_start(
                W1g[2][:], hilod[26 * 128 : 64 * 128, :], transpose=True)

        s2 = scratch.tile([128, nt * D], F32, name=f"s2{g}", tag=f"s2{g}")
        nc.scalar.activation(s2[:], xtp[:], AF.Square)
        nc.vector.tensor_reduce(biasA[:, gsl],
                                s2[:].rearrange("p (t d) -> p t d", d=D),
                                axis=mybir.AxisListType.X, op=OP.add)
        nc.vector.tensor_scalar(thrD[:, gsl], biasA[:, gsl], 0.5, -R2 / 2.0,
                                OP.mult, OP.add)
        nc.vector.tensor_scalar(biasA[:, gsl], biasA[:, gsl], -0.5, R2 / 2.0,
                                OP.mult, OP.add)
        t0 += nt

    return dict(W1g=W1g, XW=XW, XW2=XW2, SPART=SPART, MAB=MAB, biasA=biasA,
                thrD=thrD, ONEC=ONEC, IDN=IDN)


def _main(nc, tc, pools, outd, env):
    const, scratch, gpool, mpool, apool, spool = pools
    ts = bass.ts
    W1g = env["W1g"]; XW = env["XW"]; XW2 = env["XW2"]; MAB = env["MAB"]
    biasA = env["biasA"]; thrD = env["thrD"]; ONEC = env["ONEC"]
    IDN = env["IDN"]

    OUT2 = apool.tile([65, ROWS], F32, tag="OUT2")

    # tile t -> (transpose-group, tile-within-group)
    gof = []
    for g, nt in enumerate((8, 18, 38)):
        for k in range(nt):
            gof.append((g, k))

    LAG = FLAGS["lag"]
    mks = {}
    fixed_mk = None
    if not FLAGS["compare"]:
        fixed_mk = const.tile([128, ROWS], F16, tag="fixed_mk")
        nc.vector.memset(fixed_mk[:], 1.0)
    for idx in range(JT + LAG):
        if idx < JT:
            t = idx
            g, tt = gof[t]
            if FLAGS["gmm"]:
                Gt = gpool.tile([128, ROWS], F32, tag="G")
                Wt = W1g[g][0 : D + 2, ts(tt, 128)]
                for h in (0, 1):
                    cs = slice(512 * h, 512 * (h + 1))
                    nc.tensor.matmul(Gt[:, cs], Wt, MAB[0 : D + 2, cs],
                                     start=True, stop=True)
            if FLAGS["compare"]:
                mk = mpool.tile([128, ROWS], F16, tag="mk")
                if t % 2 == 0:
                    nc.vector.tensor_scalar(mk[:], Gt[:], thrD[:, t : t + 1],
                                            0.5, OP.is_ge, OP.subtract)
                else:
                    nc.scalar.activation(mk[:], Gt[:], AF.Sign,
                                         bias=biasA[:, t : t + 1])
                mks[idx] = mk
            else:
                mks[idx] = fixed_mk
        if idx >= LAG and FLAGS["passc"]:
            jt = idx - LAG
            XWsrc = XW2 if jt % 2 == 0 else XW
            xws = XWsrc[:, 65 * jt : 65 * (jt + 1)]
            mk = mks.pop(jt)
            for h in (0, 1):
                cs = slice(512 * h, 512 * (h + 1))
                nc.tensor.matmul(OUT2[:, cs], xws, mk[:, cs],
                                 start=(jt == 0), stop=(jt == JT - 1))

    # ---------------- tail ----------------
    # P = K1*OUT2 + (1+K1)*SALL; SALL[0:64] = colsum(x) via PE partition-
    # reduce of the f32 DVE partial, SALL[64] = N exactly.
    sp16 = spool.tile([128, D], F16, tag="sp16")
    nc.vector.tensor_copy(sp16[:], env["SPART"][:])
    sps = gpool.tile([128, 65], F32, tag="G")
    nc.tensor.matmul(sps[0:D, 0:1], sp16[:], ONEC[:], start=True, stop=True)
    bvec = spool.tile([65, 1], F32, tag="bvec")
    nc.vector.tensor_scalar(bvec[0:D, :], sps[0:D, 0:1], 1.0 + K1, None,
                            OP.mult)
    nc.vector.memset(bvec[D : D + 1, :], (1.0 + K1) * float(N))

    for c in range(IT):
        pc = spool.tile([65, 128], F32, tag="pc")
        if c % 2 == 0:
            nc.vector.tensor_scalar(pc[:], OUT2[:, ts(c, 128)], K1, bvec[:],
                                    OP.mult, OP.add)
        else:
            nc.scalar.activation(pc[:], OUT2[:, ts(c, 128)], AF.Identity,
                                 bias=bvec[:], scale=K1)
        pt = gpool.tile([128, 65], F32, tag="G")
        nc.tensor.transpose(pt[:], pc[:], IDN[:])
        dinv = spool.tile([128, 1], F32, tag="dinv")
        nc.vector.reciprocal(dinv[:], pt[:, D : D + 1])
        ot = spool.tile([128, D], F32, tag="ot")
        nc.vector.tensor_scalar(ot[:], pt[:, 0:D], dinv[:], None, OP.mult)
        nc.sync.dma_start(outd[ts(c, 128), :], ot[:])


def build_module(loop_n=1, scope="full"):
    nc = bacc.Bacc("TRN2", target_bir_lowering=False, debug=False,
                   num_devices=NCORES)
    xf_d = nc.dram_tensor("xf", [N, D], F32, kind="ExternalInput")
    xi_d = nc.dram_tensor("xi", [ROWS, D], F32, kind="ExternalInput")
    out_d = nc.dram_tensor("out", [ROWS, D], F32, kind="ExternalOutput")

    with tile.TileContext(nc) as tc:
        with (
            tc.tile_pool(name="const", bufs=1) as const,
            tc.tile_pool(name="scratch", bufs=2) as scratch,
            tc.tile_pool(name="gpool", bufs=3, space="PSUM") as gpool,
            tc.tile_pool(name="acc", bufs=1, space="PSUM") as apool,
            tc.tile_pool(name="mk", bufs=6) as mpool,
            tc.tile_pool(name="small", bufs=3) as spool,
            tc.tile_pool(name="dram", bufs=3, space="DRAM") as dram,
        ):
            pools = (const, scratch, gpool, mpool, apool, spool)
            args = (nc, tc, pools, xf_d.ap(), xi_d.ap(), out_d.ap(), dram)
            if scope == "pre":
                with tc.For_i(0, loop_n) as _:
                    _pre(*args)
            elif scope == "main":
                env = _pre(*args)
                with tc.For_i(0, loop_n) as _:
                    _main(nc, tc, pools, out_d.ap(), env)
            elif loop_n == 1:
                env = _pre(*args)
                _main(nc, tc, pools, out_d.ap(), env)
            else:
                with tc.For_i(0, loop_n) as _:
                    env = _pre(*args)
                    _main(nc, tc, pools, out_d.ap(), env)
    nc.finalize()
    return nc


_module_cache = {}


def _get_module(loop_n=1):
    if loop_n not in _module_cache:
        _module_cache[loop_n] = build_module(loop_n)
    return _module_cache[loop_n]


def kernel(x, adj=None):
    x = np.ascontiguousarray(np.asarray(x, dtype=np.float32))
    assert x.shape == (N, D)
    nc = _get_module(1)
    in_maps = [
        {"xf": x, "xi": x[c * ROWS : (c + 1) * ROWS]} for c in range(NCORES)
    ]
    res = run_bass_kernel_spmd(nc, in_maps, core_ids=list(range(NCORES)))
    return np.concatenate([res.results[c]["out"] for c in range(NCORES)], axis=0)
